# revision 18
# baseline (speedup 1.0000x reference)
"""MoE (top-2 of 8 experts, D=1024, FFN=4096) on 8 Trainium2 NeuronCores.

Strategy (expert-parallel, per the sharding hint):
  - Host computes the gating softmax + top-2 routing (this IS the sharding
    step: it decides which tokens go to which core).
  - Core e holds expert e's weights (bf16) and runs the FFN
    y = gelu(x @ W1 + b1) @ W2 + b2 for the tokens routed to expert e,
    capacity-padded to C tokens, activations streamed as [D, C] so the
    contraction dim always sits on SBUF partitions (no transposes on device).
  - Host scatter-adds the combine-weighted expert outputs back into the
    full [B, S, D] output.

All matmuls run in bf16 with fp32 PSUM accumulation; bias+gelu epilogues on
the scalar engine read PSUM directly.

Perf notes (v3):
  - All inputs are host-staged into partition-major blocked layouts so every
    DMA descriptor covers a 4-16KB contiguous run per partition (the naive
    [D, C] layout yields 0.5-1KB runs and ~60-110 GB/s; blocked gets near
    the ~358 GB/s HBM line rate). This pulls the first-matmul gate from
    ~14.5us to ~11.3us.
  - The PE HAM clock gate holds the array at 1.2GHz until it sees ~3.4us of
    sustained matmul activity, and re-throttles after idle windows. Dummy
    warmup matmuls bridge the DMA wait so every real matmul runs at 2.4GHz.
  - x blocks 1-3 share one SBUF slot (tag chain): each block's DMA waits for
    the previous block's last GEMM1 read, so the loads self-schedule into
    quiet DMA windows instead of contending with the critical startup.
  - y leaves the device as bf16 (rel-err budget 2e-2, this adds ~2e-4) and
    the last GEMM2 group is split so the final act+store tail is short.
"""

import math

import numpy as np
import ml_dtypes

D_MODEL = 1024
FFN_HIDDEN = 4096
N_EXPERTS = 8
TOP_K = 2
CAPACITY = 2048          # per-expert token capacity (mean load 2048); the rare
                         # overflow tokens take the exact host fallback path
CBLK = 512
NBLK = CAPACITY // CBLK  # 4 c-blocks
P = 128                  # SBUF partitions
DC = D_MODEL // P        # 8 d-chunks of 128
HC = FFN_HIDDEN // P     # 32 h-chunks of 128
W1_SLICES = [128, 128, 256, 256, 256, 512, 512, 1024, 1024]
assert sum(W1_SLICES) == FFN_HIDDEN

BF16 = ml_dtypes.bfloat16

_ACT_FUNC = "Gelu"       # sim_check overrides to "Tanh" (CoreSim lacks Gelu)
TRACE = False            # test harness sets True to collect an NTFF profile
LAST_EXEC_NS = None
LAST_TRACE_PATH = None

N_WARM = 13              # dummy matmuls to trip the PE HAM clock gate: the
                         # 4096-cycle activity window is free-running, so only
                         # >=2 windows (~6.8us) of continuous busy GUARANTEES
                         # a fully-busy window and the 2.4GHz transition; the
                         # real stream continues the busy span seamlessly

_NC_CACHE = {}


def _build_bass():
    import concourse.bacc as bacc
    import concourse.mybir as mybir
    import concourse.tile as tile

    nc = bacc.Bacc("TRN2", target_bir_lowering=False, debug=False)
    dt = mybir.dt

    xb = [nc.dram_tensor(f"xb{i}", [P, DC, CBLK], dt.bfloat16,
                         kind="ExternalInput") for i in range(NBLK)]
    w1s = [nc.dram_tensor(f"w1_{i}", [P, DC, w], dt.bfloat16,
                          kind="ExternalInput")
           for i, w in enumerate(W1_SLICES)]
    w2s = [nc.dram_tensor(f"w2_{g}", [P, HC // 2, D_MODEL], dt.bfloat16,
                          kind="ExternalInput") for g in range(2)]
    b1 = nc.dram_tensor("b1", [P, HC], dt.float32, kind="ExternalInput")
    b2 = nc.dram_tensor("b2", [P, DC], dt.float32, kind="ExternalInput")
    yt = nc.dram_tensor("yt", [D_MODEL, CAPACITY], dt.bfloat16,
                        kind="ExternalOutput")
    ytv = yt.rearrange("(dc p) c -> p dc c", p=P)

    gelu = getattr(mybir.ActivationFunctionType, _ACT_FUNC)
    ident = mybir.ActivationFunctionType.Identity

    with tile.TileContext(nc) as tc:
        with (
            tc.tile_pool(name="wpool", bufs=1) as wpool,
            tc.tile_pool(name="bpool", bufs=1) as bpool,
            tc.tile_pool(name="xpool", bufs=1) as xpool,
            tc.tile_pool(name="hpool", bufs=1) as hpool,
            tc.tile_pool(name="ypool", bufs=4) as ypool,
            tc.tile_pool(name="ps1", bufs=4, space="PSUM") as ps1pool,
            tc.tile_pool(name="ps2", bufs=4, space="PSUM") as ps2pool,
        ):
            # --- PE clock warmup -------------------------------------------
            # Dummy matmuls over a zeroed scratch tile keep the PE busy from
            # ~6.5us (right after the NEFF preamble) until the first real
            # operands land (~11.3us), so the HAM gate is at 2.4GHz for the
            # entire real stream instead of the first ~12 matmuls running at
            # half clock and the gate re-throttling on the DMA wait.
            wsc = wpool.tile([P, 640], dt.bfloat16, tag="warm")
            nc.gpsimd.memset(wsc[:], 0.0)
            wps = ps1pool.tile([P, 512], dt.float32, tag="ps1")
            for _ in range(N_WARM):
                nc.tensor.matmul(wps[:], wsc[:, 0:128], wsc[:, 128:640],
                                 start=True, stop=True)

            # --- input staging ---------------------------------------------
            # Engine rings are FIFO, so order == need-order per ring:
            #   scalar ring: x block 0 (gates the first matmul), later the
            #                y stores;
            #   sync ring:   W1 slices (small ones first), b1, W2, b2;
            #   gpsimd ring: x blocks 1-3, each gated by the tag-chain slot
            #                release (previous block's GEMM1 done) so they
            #                never contend with the startup-critical loads.
            # The startup gate for the first real matmuls is x(b0) first half
            # (scalar ring) + W1 slice 0 (sync ring) = 1MB; the second x half
            # rides sync right behind w1_0 and lands exactly when the
            # phase-split groups need dc4-7.
            x_sb = [xpool.tile([P, DC, CBLK], dt.bfloat16, tag="x",
                               name=f"x_sb{i}")
                    for i in range(NBLK)]
            nc.scalar.dma_start(x_sb[0][:, 0:DC // 2, :],
                                xb[0][:, 0:DC // 2, :])

            w1_sb = []          # list of (col_start, tile)
            col = 0
            for si, w in enumerate(W1_SLICES):
                t = wpool.tile([P, DC, w], dt.bfloat16, tag=f"w1_{si}")
                nc.sync.dma_start(t[:], w1s[si][:, :, :])
                w1_sb.append((col, t))
                col += w
                if si == 0:
                    b1_sb = bpool.tile([P, HC], dt.float32, tag="b1")
                    nc.sync.dma_start(b1_sb[:], b1[:, :])
                    nc.sync.dma_start(x_sb[0][:, DC // 2:DC, :],
                                      xb[0][:, DC // 2:DC, :])

            for i in range(1, NBLK):
                nc.gpsimd.dma_start(x_sb[i][:], xb[i][:, :, :])

            def w1_tile(hc, dc):
                """lhsT slice [P, 128] for h-tile hc, d-chunk dc."""
                h0 = hc * P
                for (col0, t), w in zip(w1_sb, W1_SLICES):
                    if col0 <= h0 < col0 + w:
                        return t[:, dc, h0 - col0:h0 - col0 + P]
                raise AssertionError(hc)

            w2_sb = []
            for g in range(2):  # 2 halves of 16 h-chunks
                t = wpool.tile([P, HC // 2, D_MODEL], dt.bfloat16,
                               tag=f"w2_{g}")
                nc.sync.dma_start(t[:], w2s[g][:, :, :])
                w2_sb.append(t)
            b2_sb = bpool.tile([P, DC], dt.float32, tag="b2")
            nc.sync.dma_start(b2_sb[:], b2[:, :])

            # --- compute ---------------------------------------------------
            for cb in range(NBLK):
                c_off = cb * CBLK
                h_t = hpool.tile([P, HC, CBLK], dt.bfloat16, tag="h")

                # GEMM1: H1^T[h, c] = sum_d W1[d, h] * X^T[d, c]
                hc0 = 0
                if cb == 0:
                    # Phase-split the first two groups: accumulate dc0-3 as
                    # soon as the first x half + w1_0 land (1MB gate), finish
                    # dc4-7 when the second half arrives ~1.8us later. This
                    # starts the real PE stream ~1.8us earlier than waiting
                    # for the whole 1.5MB.
                    hc0 = 2
                    psA = []
                    for hc in range(hc0):
                        ps = ps1pool.tile([P, CBLK], dt.float32, tag="ps1")
                        for dc in range(DC // 2):
                            nc.tensor.matmul(
                                ps[:], w1_tile(hc, dc), x_sb[0][:, dc, :],
                                start=(dc == 0), stop=False,
                            )
                        psA.append(ps)
                    for hc in range(hc0):
                        ps = psA[hc]
                        for dc in range(DC // 2, DC):
                            nc.tensor.matmul(
                                ps[:], w1_tile(hc, dc), x_sb[0][:, dc, :],
                                start=False, stop=(dc == DC - 1),
                            )
                        nc.scalar.activation(
                            h_t[:, hc, :], ps[:], gelu,
                            bias=b1_sb[:, hc:hc + 1]
                        )
                for hc in range(hc0, HC):
                    ps = ps1pool.tile([P, CBLK], dt.float32, tag="ps1")
                    for dc in range(DC):
                        nc.tensor.matmul(
                            ps[:],
                            w1_tile(hc, dc),
                            x_sb[cb][:, dc, :],
                            start=(dc == 0),
                            stop=(dc == DC - 1),
                        )
                    nc.scalar.activation(
                        h_t[:, hc, :], ps[:], gelu, bias=b1_sb[:, hc:hc + 1]
                    )

                # GEMM2: Y^T[d, c] = sum_h W2[h, d] * H1^T[h, c]
                last = (cb == NBLK - 1)
                for dt_i in range(DC):
                    nsplit = 2 if (last and dt_i == DC - 1) else 1
                    w = CBLK // nsplit
                    for sp in range(nsplit):
                        ps2 = ps2pool.tile([P, w], dt.float32, tag="ps2")
                        for hc in range(HC):
                            nc.tensor.matmul(
                                ps2[:],
                                w2_sb[hc // 16][:, hc % 16,
                                                dt_i * P:(dt_i + 1) * P],
                                h_t[:, hc, sp * w:(sp + 1) * w],
                                start=(hc == 0),
                                stop=(hc == HC - 1),
                            )
                        y_t = ypool.tile([P, w], dt.bfloat16, tag="y")
                        nc.scalar.activation(
                            y_t[:], ps2[:], ident, bias=b2_sb[:, dt_i:dt_i + 1]
                        )
                        c0 = c_off + sp * w
                        nc.scalar.dma_start(
                            ytv[:, dt_i, c0:c0 + w], y_t[:]
                        )

    nc.compile()
    return nc


def _get_nc():
    if "nc" not in _NC_CACHE:
        _NC_CACHE["nc"] = _build_bass()
    return _NC_CACHE["nc"]


def _route(x2, w_gate):
    """fp32 gating softmax + distinct top-2, matching the reference."""
    T = x2.shape[0]
    logits = x2 @ w_gate.T                      # [T, E] fp32
    m = logits.max(1, keepdims=True)
    e = np.exp(logits - m, dtype=np.float32)
    p = e / e.sum(1, keepdims=True)
    i1 = p.argmax(1)
    pm = p.copy()
    pm[np.arange(T), i1] = -1.0
    i2 = pm.argmax(1)
    s1 = p[np.arange(T), i1]
    s2 = p[np.arange(T), i2]
    return i1, i2, s1, s2


def _host_ffn_f64(xrows, W1e, b1e, W2e, b2e):
    """Exact-math fallback FFN for capacity-overflow tokens (rare)."""
    h = xrows.astype(np.float64) @ W1e.astype(np.float64) + b1e.astype(np.float64)
    try:
        from scipy.special import erf
        g = 0.5 * h * (1.0 + erf(h / math.sqrt(2.0)))
    except ImportError:
        g = 0.5 * h * (1.0 + np.frompyfunc(math.erf, 1, 1)(h / math.sqrt(2.0)).astype(np.float64))
    return g @ W2e.astype(np.float64) + b2e.astype(np.float64)


def _stage_expert(x2_bf, idx, W1e, b1e, W2e, b2e):
    """Host-side staging into the partition-major blocked DMA layouts."""
    n = len(idx)
    xp = np.zeros((D_MODEL, CAPACITY), dtype=BF16)
    xp[:, :n] = x2_bf[idx].T
    # [cb, p, dc, c] so each x block is one DMA with 8KB/partition runs
    xblk = np.ascontiguousarray(
        xp.reshape(DC, P, NBLK, CBLK).transpose(2, 1, 0, 3)
    )
    in_map = {f"xb{i}": xblk[i] for i in range(NBLK)}

    w1r = W1e.astype(BF16).reshape(DC, P, FFN_HIDDEN)  # [dc, p, h]
    col = 0
    for si, w in enumerate(W1_SLICES):
        in_map[f"w1_{si}"] = np.ascontiguousarray(
            w1r[:, :, col:col + w].transpose(1, 0, 2)  # [p, dc, w]
        )
        col += w

    w2r = W2e.astype(BF16).reshape(HC, P, D_MODEL)     # [hc, p, d]
    for g in range(2):
        in_map[f"w2_{g}"] = np.ascontiguousarray(
            w2r[g * (HC // 2):(g + 1) * (HC // 2)].transpose(1, 0, 2)
        )

    in_map["b1"] = np.ascontiguousarray(b1e.reshape(HC, P).T)
    in_map["b2"] = np.ascontiguousarray(b2e.reshape(DC, P).T)
    return in_map


def _ensure_ntff_hook():
    """Register the axon NTFF profile hook if the image's antenv lacks it.

    Only used on TRACE=True (dev profiling) runs; never on the plain path.
    """
    import sys
    import types
    try:
        import antenv.axon_hooks  # noqa: F401
        return
    except ImportError:
        pass
    hook = None
    try:
        from trn_agent_boot.trn_boot import _ntff_profile_via_ctypes
        hook = _ntff_profile_via_ctypes("/opt/axon/libaxon_pjrt.so")
    except Exception:
        hook = None
    mod = types.ModuleType("antenv.axon_hooks")
    mod.get_axon_ntff_profile_hook = lambda: hook
    mod.set_axon_ntff_profile_hook = lambda h: None
    sys.modules["antenv.axon_hooks"] = mod
    try:
        import antenv
        antenv.axon_hooks = mod
    except Exception:
        pass


def kernel(x, w_gate, W1, b1, W2, b2):
    global LAST_EXEC_NS, LAST_TRACE_PATH
    from concourse.bass_utils import run_bass_kernel_spmd
    if TRACE:
        _ensure_ntff_hook()

    x = np.asarray(x, dtype=np.float32)
    w_gate = np.asarray(w_gate, dtype=np.float32)
    W1 = np.asarray(W1, dtype=np.float32)
    b1 = np.asarray(b1, dtype=np.float32)
    W2 = np.asarray(W2, dtype=np.float32)
    b2 = np.asarray(b2, dtype=np.float32)

    B, S, D = x.shape
    T = B * S
    x2 = np.ascontiguousarray(x.reshape(T, D))

    i1, i2, s1, s2 = _route(x2, w_gate)

    # Per-expert dispatch lists (a token appears at most once per expert).
    idx_e, w_e = [], []
    for e in range(N_EXPERTS):
        a = np.nonzero(i1 == e)[0]
        b = np.nonzero(i2 == e)[0]
        idx = np.concatenate([a, b])
        w = np.concatenate([s1[a], s2[b]]).astype(np.float32)
        idx_e.append(idx)
        w_e.append(w)

    x2_bf = x2.astype(BF16)
    in_maps = []
    overflow = []  # (expert, token_ids, weights) beyond capacity
    for e in range(N_EXPERTS):
        idx = idx_e[e]
        if len(idx) > CAPACITY:
            overflow.append((e, idx[CAPACITY:], w_e[e][CAPACITY:]))
            idx = idx[:CAPACITY]
            idx_e[e] = idx
            w_e[e] = w_e[e][:CAPACITY]
        in_maps.append(_stage_expert(x2_bf, idx, W1[e], b1[e], W2[e], b2[e]))

    nc = _get_nc()
    res = None
    for attempt in range(3):  # transient NRT device errors: retry
        try:
            res = run_bass_kernel_spmd(
                nc, in_maps, core_ids=list(range(N_EXPERTS)), trace=TRACE
            )
            break
        except Exception:
            if attempt == 2:
                raise
            import time
            time.sleep(2.0)
    LAST_EXEC_NS = res.exec_time_ns
    if res.instructions_and_trace is not None:
        LAST_TRACE_PATH = res.instructions_and_trace[1]

    out = np.zeros((T, D), dtype=np.float32)
    for e in range(N_EXPERTS):
        idx = idx_e[e]
        if len(idx) == 0:
            continue
        ye = res.results[e]["yt"][:, :len(idx)].T.astype(np.float32)  # [n_e, D]
        out[idx] += w_e[e][:, None] * ye
    for e, idx, w in overflow:
        ye = _host_ffn_f64(x2[idx], W1[e], b1[e], W2[e], b2[e])
        out[idx] += (w[:, None] * ye).astype(np.float32)

    return out.reshape(B, S, D)


# revision 19
# speedup vs baseline: 1.0085x; 1.0085x over previous
"""MoE (top-2 of 8 experts, D=1024, FFN=4096) on 8 Trainium2 NeuronCores.

Strategy (expert-parallel, per the sharding hint):
  - Host computes the gating softmax + top-2 routing (this IS the sharding
    step: it decides which tokens go to which core).
  - Core e holds expert e's weights (bf16) and runs the FFN
    y = gelu(x @ W1 + b1) @ W2 + b2 for the tokens routed to expert e,
    capacity-padded to C tokens, activations streamed as [D, C] so the
    contraction dim always sits on SBUF partitions (no transposes on device).
  - Host scatter-adds the combine-weighted expert outputs back into the
    full [B, S, D] output.

All matmuls run in bf16 with fp32 PSUM accumulation; bias+gelu epilogues on
the scalar engine read PSUM directly.

Perf notes (v3):
  - All inputs are host-staged into partition-major blocked layouts so every
    DMA descriptor covers a 4-16KB contiguous run per partition (the naive
    [D, C] layout yields 0.5-1KB runs and ~60-110 GB/s; blocked gets near
    the ~358 GB/s HBM line rate). This pulls the first-matmul gate from
    ~14.5us to ~11.3us.
  - The PE HAM clock gate holds the array at 1.2GHz until it sees ~3.4us of
    sustained matmul activity, and re-throttles after idle windows. Dummy
    warmup matmuls bridge the DMA wait so every real matmul runs at 2.4GHz.
  - x blocks 1-3 share one SBUF slot (tag chain): each block's DMA waits for
    the previous block's last GEMM1 read, so the loads self-schedule into
    quiet DMA windows instead of contending with the critical startup.
  - y leaves the device as bf16 (rel-err budget 2e-2, this adds ~2e-4) and
    the last GEMM2 group is split so the final act+store tail is short.
"""

import math

import numpy as np
import ml_dtypes

D_MODEL = 1024
FFN_HIDDEN = 4096
N_EXPERTS = 8
TOP_K = 2
CAPACITY = 2048          # per-expert token capacity (mean load 2048); the rare
                         # overflow tokens take the exact host fallback path
CBLK = 512
NBLK = CAPACITY // CBLK  # 4 c-blocks
P = 128                  # SBUF partitions
DC = D_MODEL // P        # 8 d-chunks of 128
HC = FFN_HIDDEN // P     # 32 h-chunks of 128
W1_SLICES = [256, 128, 128, 256, 256, 512, 512, 1024, 1024]
assert sum(W1_SLICES) == FFN_HIDDEN

BF16 = ml_dtypes.bfloat16

_ACT_FUNC = "Gelu"       # sim_check overrides to "Tanh" (CoreSim lacks Gelu)
TRACE = False            # test harness sets True to collect an NTFF profile
LAST_EXEC_NS = None
LAST_TRACE_PATH = None

N_WARM = 13              # dummy matmuls to trip the PE HAM clock gate: the
                         # 4096-cycle activity window is free-running, so only
                         # >=2 windows (~6.8us) of continuous busy GUARANTEES
                         # a fully-busy window and the 2.4GHz transition; the
                         # real stream continues the busy span seamlessly

_NC_CACHE = {}


def _build_bass():
    import concourse.bacc as bacc
    import concourse.mybir as mybir
    import concourse.tile as tile

    nc = bacc.Bacc("TRN2", target_bir_lowering=False, debug=False)
    dt = mybir.dt

    xb = [nc.dram_tensor(f"xb{i}", [P, DC, CBLK], dt.bfloat16,
                         kind="ExternalInput") for i in range(NBLK)]
    w1s = [nc.dram_tensor(f"w1_{i}", [P, DC, w], dt.bfloat16,
                          kind="ExternalInput")
           for i, w in enumerate(W1_SLICES)]
    w2s = [nc.dram_tensor(f"w2_{g}", [P, HC // 2, D_MODEL], dt.bfloat16,
                          kind="ExternalInput") for g in range(2)]
    b1 = nc.dram_tensor("b1", [P, HC], dt.float32, kind="ExternalInput")
    b2 = nc.dram_tensor("b2", [P, DC], dt.float32, kind="ExternalInput")
    yt = nc.dram_tensor("yt", [D_MODEL, CAPACITY], dt.bfloat16,
                        kind="ExternalOutput")
    ytv = yt.rearrange("(dc p) c -> p dc c", p=P)

    gelu = getattr(mybir.ActivationFunctionType, _ACT_FUNC)
    ident = mybir.ActivationFunctionType.Identity

    with tile.TileContext(nc) as tc:
        with (
            tc.tile_pool(name="wpool", bufs=1) as wpool,
            tc.tile_pool(name="bpool", bufs=1) as bpool,
            tc.tile_pool(name="xpool", bufs=1) as xpool,
            tc.tile_pool(name="hpool", bufs=1) as hpool,
            tc.tile_pool(name="ypool", bufs=4) as ypool,
            tc.tile_pool(name="ps1", bufs=4, space="PSUM") as ps1pool,
            tc.tile_pool(name="ps2", bufs=4, space="PSUM") as ps2pool,
        ):
            # --- PE clock warmup -------------------------------------------
            # Dummy matmuls over a zeroed scratch tile keep the PE busy from
            # ~6.5us (right after the NEFF preamble) until the first real
            # operands land (~11.3us), so the HAM gate is at 2.4GHz for the
            # entire real stream instead of the first ~12 matmuls running at
            # half clock and the gate re-throttling on the DMA wait.
            wsc = wpool.tile([P, 640], dt.bfloat16, tag="warm")
            nc.gpsimd.memset(wsc[:], 0.0)
            wps = ps1pool.tile([P, 512], dt.float32, tag="ps1")
            for _ in range(N_WARM):
                nc.tensor.matmul(wps[:], wsc[:, 0:128], wsc[:, 128:640],
                                 start=True, stop=True)

            # --- input staging ---------------------------------------------
            # Engine rings are FIFO, so order == need-order per ring:
            #   scalar ring: x block 0 (gates the first matmul), later the
            #                y stores;
            #   sync ring:   W1 slices (small ones first), b1, W2, b2;
            #   gpsimd ring: x blocks 1-3, each gated by the tag-chain slot
            #                release (previous block's GEMM1 done) so they
            #                never contend with the startup-critical loads.
            # The startup gate for the first real matmuls is x(b0) first half
            # (scalar ring) + W1 slice 0 (sync ring) = 1MB; the second x half
            # rides sync right behind w1_0 and lands exactly when the
            # phase-split groups need dc4-7.
            x_sb = [xpool.tile([P, DC, CBLK], dt.bfloat16, tag="x",
                               name=f"x_sb{i}")
                    for i in range(NBLK)]
            nc.scalar.dma_start(x_sb[0][:, 0:DC // 2, :],
                                xb[0][:, 0:DC // 2, :])

            w1_sb = []          # list of (col_start, tile)
            col = 0
            for si, w in enumerate(W1_SLICES):
                t = wpool.tile([P, DC, w], dt.bfloat16, tag=f"w1_{si}")
                nc.sync.dma_start(t[:], w1s[si][:, :, :])
                w1_sb.append((col, t))
                col += w
                if si == 0:
                    b1_sb = bpool.tile([P, HC], dt.float32, tag="b1")
                    nc.sync.dma_start(b1_sb[:], b1[:, :])
                    nc.sync.dma_start(x_sb[0][:, DC // 2:DC, :],
                                      xb[0][:, DC // 2:DC, :])

            for i in range(1, NBLK):
                nc.gpsimd.dma_start(x_sb[i][:], xb[i][:, :, :])

            def w1_tile(hc, dc):
                """lhsT slice [P, 128] for h-tile hc, d-chunk dc."""
                h0 = hc * P
                for (col0, t), w in zip(w1_sb, W1_SLICES):
                    if col0 <= h0 < col0 + w:
                        return t[:, dc, h0 - col0:h0 - col0 + P]
                raise AssertionError(hc)

            w2_sb = []
            for g in range(2):  # 2 halves of 16 h-chunks
                t = wpool.tile([P, HC // 2, D_MODEL], dt.bfloat16,
                               tag=f"w2_{g}")
                nc.sync.dma_start(t[:], w2s[g][:, :, :])
                w2_sb.append(t)
            b2_sb = bpool.tile([P, DC], dt.float32, tag="b2")
            nc.sync.dma_start(b2_sb[:], b2[:, :])

            # --- compute ---------------------------------------------------
            for cb in range(NBLK):
                c_off = cb * CBLK
                h_t = hpool.tile([P, HC, CBLK], dt.bfloat16, tag="h")

                # GEMM1: H1^T[h, c] = sum_d W1[d, h] * X^T[d, c]
                hc0 = 0
                if cb == 0:
                    # Phase-split the first two groups: accumulate dc0-3 as
                    # soon as the first x half + w1_0 land (1MB gate), finish
                    # dc4-7 when the second half arrives ~1.8us later. This
                    # starts the real PE stream ~1.8us earlier than waiting
                    # for the whole 1.5MB.
                    hc0 = 2
                    psA = []
                    for hc in range(hc0):
                        ps = ps1pool.tile([P, CBLK], dt.float32, tag="ps1")
                        for dc in range(DC // 2):
                            nc.tensor.matmul(
                                ps[:], w1_tile(hc, dc), x_sb[0][:, dc, :],
                                start=(dc == 0), stop=False,
                            )
                        psA.append(ps)
                    for hc in range(hc0):
                        ps = psA[hc]
                        for dc in range(DC // 2, DC):
                            nc.tensor.matmul(
                                ps[:], w1_tile(hc, dc), x_sb[0][:, dc, :],
                                start=False, stop=(dc == DC - 1),
                            )
                        nc.scalar.activation(
                            h_t[:, hc, :], ps[:], gelu,
                            bias=b1_sb[:, hc:hc + 1]
                        )
                for hc in range(hc0, HC):
                    ps = ps1pool.tile([P, CBLK], dt.float32, tag="ps1")
                    for dc in range(DC):
                        nc.tensor.matmul(
                            ps[:],
                            w1_tile(hc, dc),
                            x_sb[cb][:, dc, :],
                            start=(dc == 0),
                            stop=(dc == DC - 1),
                        )
                    nc.scalar.activation(
                        h_t[:, hc, :], ps[:], gelu, bias=b1_sb[:, hc:hc + 1]
                    )

                # GEMM2: Y^T[d, c] = sum_h W2[h, d] * H1^T[h, c]
                last = (cb == NBLK - 1)
                for dt_i in range(DC):
                    nsplit = 2 if (last and dt_i == DC - 1) else 1
                    w = CBLK // nsplit
                    for sp in range(nsplit):
                        ps2 = ps2pool.tile([P, w], dt.float32, tag="ps2")
                        for hc in range(HC):
                            nc.tensor.matmul(
                                ps2[:],
                                w2_sb[hc // 16][:, hc % 16,
                                                dt_i * P:(dt_i + 1) * P],
                                h_t[:, hc, sp * w:(sp + 1) * w],
                                start=(hc == 0),
                                stop=(hc == HC - 1),
                            )
                        y_t = ypool.tile([P, w], dt.bfloat16, tag="y")
                        nc.scalar.activation(
                            y_t[:], ps2[:], ident, bias=b2_sb[:, dt_i:dt_i + 1]
                        )
                        c0 = c_off + sp * w
                        nc.scalar.dma_start(
                            ytv[:, dt_i, c0:c0 + w], y_t[:]
                        )

    nc.compile()
    return nc


def _get_nc():
    if "nc" not in _NC_CACHE:
        _NC_CACHE["nc"] = _build_bass()
    return _NC_CACHE["nc"]


def _route(x2, w_gate):
    """fp32 gating softmax + distinct top-2, matching the reference."""
    T = x2.shape[0]
    logits = x2 @ w_gate.T                      # [T, E] fp32
    m = logits.max(1, keepdims=True)
    e = np.exp(logits - m, dtype=np.float32)
    p = e / e.sum(1, keepdims=True)
    i1 = p.argmax(1)
    pm = p.copy()
    pm[np.arange(T), i1] = -1.0
    i2 = pm.argmax(1)
    s1 = p[np.arange(T), i1]
    s2 = p[np.arange(T), i2]
    return i1, i2, s1, s2


def _host_ffn_f64(xrows, W1e, b1e, W2e, b2e):
    """Exact-math fallback FFN for capacity-overflow tokens (rare)."""
    h = xrows.astype(np.float64) @ W1e.astype(np.float64) + b1e.astype(np.float64)
    try:
        from scipy.special import erf
        g = 0.5 * h * (1.0 + erf(h / math.sqrt(2.0)))
    except ImportError:
        g = 0.5 * h * (1.0 + np.frompyfunc(math.erf, 1, 1)(h / math.sqrt(2.0)).astype(np.float64))
    return g @ W2e.astype(np.float64) + b2e.astype(np.float64)


def _stage_expert(x2_bf, idx, W1e, b1e, W2e, b2e):
    """Host-side staging into the partition-major blocked DMA layouts."""
    n = len(idx)
    xp = np.zeros((D_MODEL, CAPACITY), dtype=BF16)
    xp[:, :n] = x2_bf[idx].T
    # [cb, p, dc, c] so each x block is one DMA with 8KB/partition runs
    xblk = np.ascontiguousarray(
        xp.reshape(DC, P, NBLK, CBLK).transpose(2, 1, 0, 3)
    )
    in_map = {f"xb{i}": xblk[i] for i in range(NBLK)}

    w1r = W1e.astype(BF16).reshape(DC, P, FFN_HIDDEN)  # [dc, p, h]
    col = 0
    for si, w in enumerate(W1_SLICES):
        in_map[f"w1_{si}"] = np.ascontiguousarray(
            w1r[:, :, col:col + w].transpose(1, 0, 2)  # [p, dc, w]
        )
        col += w

    w2r = W2e.astype(BF16).reshape(HC, P, D_MODEL)     # [hc, p, d]
    for g in range(2):
        in_map[f"w2_{g}"] = np.ascontiguousarray(
            w2r[g * (HC // 2):(g + 1) * (HC // 2)].transpose(1, 0, 2)
        )

    in_map["b1"] = np.ascontiguousarray(b1e.reshape(HC, P).T)
    in_map["b2"] = np.ascontiguousarray(b2e.reshape(DC, P).T)
    return in_map


def _ensure_ntff_hook():
    """Register the axon NTFF profile hook if the image's antenv lacks it.

    Only used on TRACE=True (dev profiling) runs; never on the plain path.
    """
    import sys
    import types
    try:
        import antenv.axon_hooks  # noqa: F401
        return
    except ImportError:
        pass
    hook = None
    try:
        from trn_agent_boot.trn_boot import _ntff_profile_via_ctypes
        hook = _ntff_profile_via_ctypes("/opt/axon/libaxon_pjrt.so")
    except Exception:
        hook = None
    mod = types.ModuleType("antenv.axon_hooks")
    mod.get_axon_ntff_profile_hook = lambda: hook
    mod.set_axon_ntff_profile_hook = lambda h: None
    sys.modules["antenv.axon_hooks"] = mod
    try:
        import antenv
        antenv.axon_hooks = mod
    except Exception:
        pass


def kernel(x, w_gate, W1, b1, W2, b2):
    global LAST_EXEC_NS, LAST_TRACE_PATH
    from concourse.bass_utils import run_bass_kernel_spmd
    if TRACE:
        _ensure_ntff_hook()

    x = np.asarray(x, dtype=np.float32)
    w_gate = np.asarray(w_gate, dtype=np.float32)
    W1 = np.asarray(W1, dtype=np.float32)
    b1 = np.asarray(b1, dtype=np.float32)
    W2 = np.asarray(W2, dtype=np.float32)
    b2 = np.asarray(b2, dtype=np.float32)

    B, S, D = x.shape
    T = B * S
    x2 = np.ascontiguousarray(x.reshape(T, D))

    i1, i2, s1, s2 = _route(x2, w_gate)

    # Per-expert dispatch lists (a token appears at most once per expert).
    idx_e, w_e = [], []
    for e in range(N_EXPERTS):
        a = np.nonzero(i1 == e)[0]
        b = np.nonzero(i2 == e)[0]
        idx = np.concatenate([a, b])
        w = np.concatenate([s1[a], s2[b]]).astype(np.float32)
        idx_e.append(idx)
        w_e.append(w)

    x2_bf = x2.astype(BF16)
    in_maps = []
    overflow = []  # (expert, token_ids, weights) beyond capacity
    for e in range(N_EXPERTS):
        idx = idx_e[e]
        if len(idx) > CAPACITY:
            overflow.append((e, idx[CAPACITY:], w_e[e][CAPACITY:]))
            idx = idx[:CAPACITY]
            idx_e[e] = idx
            w_e[e] = w_e[e][:CAPACITY]
        in_maps.append(_stage_expert(x2_bf, idx, W1[e], b1[e], W2[e], b2[e]))

    nc = _get_nc()
    res = None
    for attempt in range(3):  # transient NRT device errors: retry
        try:
            res = run_bass_kernel_spmd(
                nc, in_maps, core_ids=list(range(N_EXPERTS)), trace=TRACE
            )
            break
        except Exception:
            if attempt == 2:
                raise
            import time
            time.sleep(2.0)
    LAST_EXEC_NS = res.exec_time_ns
    if res.instructions_and_trace is not None:
        LAST_TRACE_PATH = res.instructions_and_trace[1]

    out = np.zeros((T, D), dtype=np.float32)
    for e in range(N_EXPERTS):
        idx = idx_e[e]
        if len(idx) == 0:
            continue
        ye = res.results[e]["yt"][:, :len(idx)].T.astype(np.float32)  # [n_e, D]
        out[idx] += w_e[e][:, None] * ye
    for e, idx, w in overflow:
        ye = _host_ffn_f64(x2[idx], W1[e], b1[e], W2[e], b2[e])
        out[idx] += (w[:, None] * ye).astype(np.float32)

    return out.reshape(B, S, D)


# revision 20
# speedup vs baseline: 1.0090x; 1.0005x over previous
"""MoE (top-2 of 8 experts, D=1024, FFN=4096) on 8 Trainium2 NeuronCores.

Strategy (expert-parallel, per the sharding hint):
  - Host computes the gating softmax + top-2 routing (this IS the sharding
    step: it decides which tokens go to which core).
  - Core e holds expert e's weights (bf16) and runs the FFN
    y = gelu(x @ W1 + b1) @ W2 + b2 for the tokens routed to expert e,
    capacity-padded to C tokens, activations streamed as [D, C] so the
    contraction dim always sits on SBUF partitions (no transposes on device).
  - Host scatter-adds the combine-weighted expert outputs back into the
    full [B, S, D] output.

All matmuls run in bf16 with fp32 PSUM accumulation; bias+gelu epilogues on
the scalar engine read PSUM directly.

Perf notes (final; ~460.6us vs 470.3us baseline, PE at the bf16 floor):
  - All inputs are host-staged into partition-major blocked layouts so every
    DMA descriptor covers a 4-16KB contiguous run per partition (the naive
    [D, C] layout yields 0.5-1KB runs and ~60-110 GB/s; blocked runs near
    line rate).
  - The PE HAM clock gate holds the array at 1.2GHz until a free-running
    ~3.4us activity window is FULLY busy (guaranteed only after ~6.8us of
    continuous work), and re-throttles after idle-dominant windows. Dummy
    warmup matmuls bridge the preamble-to-data window so the real stream
    runs entirely at 2.4GHz.
  - The first-matmul gate is 1MB (x block-0 first half on the scalar ring +
    W1 slice 0 on the sync ring); GEMM1's first two groups are phase-split
    (dc0-3 accumulate with stop=False, dc4-7 finish when the second x half
    lands) so real work starts at the 1MB mark, not 1.5MB.
  - W1 slice widths are shaped so the DMA supply curve stays ahead of PE
    consumption for every group: the trace shows a gapless 442us stream.
  - x blocks 1-3 share one SBUF slot (tag chain): each block's DMA waits for
    the previous block's last GEMM1 read, so the loads self-schedule into
    quiet DMA windows instead of contending with the critical startup.
  - y leaves the device as bf16 (rel-err budget 2e-2, this adds ~2e-4) and
    the last GEMM2 group is split 2xN=256 so the final act+store tail after
    the last matmul is short.
  - Known immovable costs: ~6us NEFF preamble, ~5us NX dispatch overhead
    (2.6ns/matmul, N capped at 512 by the PSUM bank), an environmental
    ~216ns stall every 10.79us (~8.9us total, present in every trace incl.
    the unmodified baseline), and the ~4.7us tail drain/EVSEM barrier.
"""

import math

import numpy as np
import ml_dtypes

D_MODEL = 1024
FFN_HIDDEN = 4096
N_EXPERTS = 8
TOP_K = 2
CAPACITY = 2048          # per-expert token capacity (mean load 2048); the rare
                         # overflow tokens take the exact host fallback path
CBLK = 512
NBLK = CAPACITY // CBLK  # 4 c-blocks
P = 128                  # SBUF partitions
DC = D_MODEL // P        # 8 d-chunks of 128
HC = FFN_HIDDEN // P     # 32 h-chunks of 128
W1_SLICES = [256, 128, 128, 256, 256, 512, 512, 1024, 1024]
assert sum(W1_SLICES) == FFN_HIDDEN

BF16 = ml_dtypes.bfloat16

_ACT_FUNC = "Gelu"       # sim_check overrides to "Tanh" (CoreSim lacks Gelu)
TRACE = False            # test harness sets True to collect an NTFF profile
LAST_EXEC_NS = None
LAST_TRACE_PATH = None

N_WARM = 13              # dummy matmuls to trip the PE HAM clock gate: the
                         # 4096-cycle activity window is free-running, so only
                         # >=2 windows (~6.8us) of continuous busy GUARANTEES
                         # a fully-busy window and the 2.4GHz transition; the
                         # real stream continues the busy span seamlessly

_NC_CACHE = {}


def _build_bass():
    import concourse.bacc as bacc
    import concourse.mybir as mybir
    import concourse.tile as tile

    nc = bacc.Bacc("TRN2", target_bir_lowering=False, debug=False)
    dt = mybir.dt

    xb = [nc.dram_tensor(f"xb{i}", [P, DC, CBLK], dt.bfloat16,
                         kind="ExternalInput") for i in range(NBLK)]
    w1s = [nc.dram_tensor(f"w1_{i}", [P, DC, w], dt.bfloat16,
                          kind="ExternalInput")
           for i, w in enumerate(W1_SLICES)]
    w2s = [nc.dram_tensor(f"w2_{g}", [P, HC // 2, D_MODEL], dt.bfloat16,
                          kind="ExternalInput") for g in range(2)]
    b1 = nc.dram_tensor("b1", [P, HC], dt.float32, kind="ExternalInput")
    b2 = nc.dram_tensor("b2", [P, DC], dt.float32, kind="ExternalInput")
    yt = nc.dram_tensor("yt", [D_MODEL, CAPACITY], dt.bfloat16,
                        kind="ExternalOutput")
    ytv = yt.rearrange("(dc p) c -> p dc c", p=P)

    gelu = getattr(mybir.ActivationFunctionType, _ACT_FUNC)
    ident = mybir.ActivationFunctionType.Identity

    with tile.TileContext(nc) as tc:
        with (
            tc.tile_pool(name="wpool", bufs=1) as wpool,
            tc.tile_pool(name="bpool", bufs=1) as bpool,
            tc.tile_pool(name="xpool", bufs=1) as xpool,
            tc.tile_pool(name="hpool", bufs=1) as hpool,
            tc.tile_pool(name="ypool", bufs=4) as ypool,
            tc.tile_pool(name="ps1", bufs=4, space="PSUM") as ps1pool,
            tc.tile_pool(name="ps2", bufs=4, space="PSUM") as ps2pool,
        ):
            # --- PE clock warmup -------------------------------------------
            # Dummy matmuls over a zeroed scratch tile keep the PE busy from
            # ~6.5us (right after the NEFF preamble) until the first real
            # operands land (~11.3us), so the HAM gate is at 2.4GHz for the
            # entire real stream instead of the first ~12 matmuls running at
            # half clock and the gate re-throttling on the DMA wait.
            wsc = wpool.tile([P, 640], dt.bfloat16, tag="warm")
            nc.gpsimd.memset(wsc[:], 0.0)
            wps = ps1pool.tile([P, 512], dt.float32, tag="ps1")
            for _ in range(N_WARM):
                nc.tensor.matmul(wps[:], wsc[:, 0:128], wsc[:, 128:640],
                                 start=True, stop=True)

            # --- input staging ---------------------------------------------
            # Engine rings are FIFO, so order == need-order per ring:
            #   scalar ring: x block 0 (gates the first matmul), later the
            #                y stores;
            #   sync ring:   W1 slices (small ones first), b1, W2, b2;
            #   gpsimd ring: x blocks 1-3, each gated by the tag-chain slot
            #                release (previous block's GEMM1 done) so they
            #                never contend with the startup-critical loads.
            # The startup gate for the first real matmuls is x(b0) first half
            # (scalar ring) + W1 slice 0 (sync ring) = 1MB; the second x half
            # rides sync right behind w1_0 and lands exactly when the
            # phase-split groups need dc4-7.
            x_sb = [xpool.tile([P, DC, CBLK], dt.bfloat16, tag="x",
                               name=f"x_sb{i}")
                    for i in range(NBLK)]
            nc.scalar.dma_start(x_sb[0][:, 0:DC // 2, :],
                                xb[0][:, 0:DC // 2, :])

            w1_sb = []          # list of (col_start, tile)
            col = 0
            for si, w in enumerate(W1_SLICES):
                t = wpool.tile([P, DC, w], dt.bfloat16, tag=f"w1_{si}")
                nc.sync.dma_start(t[:], w1s[si][:, :, :])
                w1_sb.append((col, t))
                col += w
                if si == 0:
                    b1_sb = bpool.tile([P, HC], dt.float32, tag="b1")
                    nc.sync.dma_start(b1_sb[:], b1[:, :])
                    nc.sync.dma_start(x_sb[0][:, DC // 2:DC, :],
                                      xb[0][:, DC // 2:DC, :])

            for i in range(1, NBLK):
                nc.gpsimd.dma_start(x_sb[i][:], xb[i][:, :, :])

            def w1_tile(hc, dc):
                """lhsT slice [P, 128] for h-tile hc, d-chunk dc."""
                h0 = hc * P
                for (col0, t), w in zip(w1_sb, W1_SLICES):
                    if col0 <= h0 < col0 + w:
                        return t[:, dc, h0 - col0:h0 - col0 + P]
                raise AssertionError(hc)

            w2_sb = []
            for g in range(2):  # 2 halves of 16 h-chunks
                t = wpool.tile([P, HC // 2, D_MODEL], dt.bfloat16,
                               tag=f"w2_{g}")
                nc.sync.dma_start(t[:], w2s[g][:, :, :])
                w2_sb.append(t)
            b2_sb = bpool.tile([P, DC], dt.float32, tag="b2")
            nc.sync.dma_start(b2_sb[:], b2[:, :])

            # --- compute ---------------------------------------------------
            for cb in range(NBLK):
                c_off = cb * CBLK
                h_t = hpool.tile([P, HC, CBLK], dt.bfloat16, tag="h")

                # GEMM1: H1^T[h, c] = sum_d W1[d, h] * X^T[d, c]
                hc0 = 0
                if cb == 0:
                    # Phase-split the first two groups: accumulate dc0-3 as
                    # soon as the first x half + w1_0 land (1MB gate), finish
                    # dc4-7 when the second half arrives ~1.8us later. This
                    # starts the real PE stream ~1.8us earlier than waiting
                    # for the whole 1.5MB.
                    hc0 = 2
                    psA = []
                    for hc in range(hc0):
                        ps = ps1pool.tile([P, CBLK], dt.float32, tag="ps1")
                        for dc in range(DC // 2):
                            nc.tensor.matmul(
                                ps[:], w1_tile(hc, dc), x_sb[0][:, dc, :],
                                start=(dc == 0), stop=False,
                            )
                        psA.append(ps)
                    for hc in range(hc0):
                        ps = psA[hc]
                        for dc in range(DC // 2, DC):
                            nc.tensor.matmul(
                                ps[:], w1_tile(hc, dc), x_sb[0][:, dc, :],
                                start=False, stop=(dc == DC - 1),
                            )
                        nc.scalar.activation(
                            h_t[:, hc, :], ps[:], gelu,
                            bias=b1_sb[:, hc:hc + 1]
                        )
                for hc in range(hc0, HC):
                    ps = ps1pool.tile([P, CBLK], dt.float32, tag="ps1")
                    for dc in range(DC):
                        nc.tensor.matmul(
                            ps[:],
                            w1_tile(hc, dc),
                            x_sb[cb][:, dc, :],
                            start=(dc == 0),
                            stop=(dc == DC - 1),
                        )
                    nc.scalar.activation(
                        h_t[:, hc, :], ps[:], gelu, bias=b1_sb[:, hc:hc + 1]
                    )

                # GEMM2: Y^T[d, c] = sum_h W2[h, d] * H1^T[h, c]
                last = (cb == NBLK - 1)
                for dt_i in range(DC):
                    nsplit = 2 if (last and dt_i == DC - 1) else 1
                    w = CBLK // nsplit
                    for sp in range(nsplit):
                        ps2 = ps2pool.tile([P, w], dt.float32, tag="ps2")
                        for hc in range(HC):
                            nc.tensor.matmul(
                                ps2[:],
                                w2_sb[hc // 16][:, hc % 16,
                                                dt_i * P:(dt_i + 1) * P],
                                h_t[:, hc, sp * w:(sp + 1) * w],
                                start=(hc == 0),
                                stop=(hc == HC - 1),
                            )
                        y_t = ypool.tile([P, w], dt.bfloat16, tag="y")
                        nc.scalar.activation(
                            y_t[:], ps2[:], ident, bias=b2_sb[:, dt_i:dt_i + 1]
                        )
                        c0 = c_off + sp * w
                        nc.scalar.dma_start(
                            ytv[:, dt_i, c0:c0 + w], y_t[:]
                        )

    nc.compile()
    return nc


def _get_nc():
    if "nc" not in _NC_CACHE:
        _NC_CACHE["nc"] = _build_bass()
    return _NC_CACHE["nc"]


def _route(x2, w_gate):
    """fp32 gating softmax + distinct top-2, matching the reference."""
    T = x2.shape[0]
    logits = x2 @ w_gate.T                      # [T, E] fp32
    m = logits.max(1, keepdims=True)
    e = np.exp(logits - m, dtype=np.float32)
    p = e / e.sum(1, keepdims=True)
    i1 = p.argmax(1)
    pm = p.copy()
    pm[np.arange(T), i1] = -1.0
    i2 = pm.argmax(1)
    s1 = p[np.arange(T), i1]
    s2 = p[np.arange(T), i2]
    return i1, i2, s1, s2


def _host_ffn_f64(xrows, W1e, b1e, W2e, b2e):
    """Exact-math fallback FFN for capacity-overflow tokens (rare)."""
    h = xrows.astype(np.float64) @ W1e.astype(np.float64) + b1e.astype(np.float64)
    try:
        from scipy.special import erf
        g = 0.5 * h * (1.0 + erf(h / math.sqrt(2.0)))
    except ImportError:
        g = 0.5 * h * (1.0 + np.frompyfunc(math.erf, 1, 1)(h / math.sqrt(2.0)).astype(np.float64))
    return g @ W2e.astype(np.float64) + b2e.astype(np.float64)


def _stage_expert(x2_bf, idx, W1e, b1e, W2e, b2e):
    """Host-side staging into the partition-major blocked DMA layouts."""
    n = len(idx)
    xp = np.zeros((D_MODEL, CAPACITY), dtype=BF16)
    xp[:, :n] = x2_bf[idx].T
    # [cb, p, dc, c] so each x block is one DMA with 8KB/partition runs
    xblk = np.ascontiguousarray(
        xp.reshape(DC, P, NBLK, CBLK).transpose(2, 1, 0, 3)
    )
    in_map = {f"xb{i}": xblk[i] for i in range(NBLK)}

    w1r = W1e.astype(BF16).reshape(DC, P, FFN_HIDDEN)  # [dc, p, h]
    col = 0
    for si, w in enumerate(W1_SLICES):
        in_map[f"w1_{si}"] = np.ascontiguousarray(
            w1r[:, :, col:col + w].transpose(1, 0, 2)  # [p, dc, w]
        )
        col += w

    w2r = W2e.astype(BF16).reshape(HC, P, D_MODEL)     # [hc, p, d]
    for g in range(2):
        in_map[f"w2_{g}"] = np.ascontiguousarray(
            w2r[g * (HC // 2):(g + 1) * (HC // 2)].transpose(1, 0, 2)
        )

    in_map["b1"] = np.ascontiguousarray(b1e.reshape(HC, P).T)
    in_map["b2"] = np.ascontiguousarray(b2e.reshape(DC, P).T)
    return in_map


def _ensure_ntff_hook():
    """Register the axon NTFF profile hook if the image's antenv lacks it.

    Only used on TRACE=True (dev profiling) runs; never on the plain path.
    """
    import sys
    import types
    try:
        import antenv.axon_hooks  # noqa: F401
        return
    except ImportError:
        pass
    hook = None
    try:
        from trn_agent_boot.trn_boot import _ntff_profile_via_ctypes
        hook = _ntff_profile_via_ctypes("/opt/axon/libaxon_pjrt.so")
    except Exception:
        hook = None
    mod = types.ModuleType("antenv.axon_hooks")
    mod.get_axon_ntff_profile_hook = lambda: hook
    mod.set_axon_ntff_profile_hook = lambda h: None
    sys.modules["antenv.axon_hooks"] = mod
    try:
        import antenv
        antenv.axon_hooks = mod
    except Exception:
        pass


def kernel(x, w_gate, W1, b1, W2, b2):
    global LAST_EXEC_NS, LAST_TRACE_PATH
    from concourse.bass_utils import run_bass_kernel_spmd
    if TRACE:
        _ensure_ntff_hook()

    x = np.asarray(x, dtype=np.float32)
    w_gate = np.asarray(w_gate, dtype=np.float32)
    W1 = np.asarray(W1, dtype=np.float32)
    b1 = np.asarray(b1, dtype=np.float32)
    W2 = np.asarray(W2, dtype=np.float32)
    b2 = np.asarray(b2, dtype=np.float32)

    B, S, D = x.shape
    T = B * S
    x2 = np.ascontiguousarray(x.reshape(T, D))

    i1, i2, s1, s2 = _route(x2, w_gate)

    # Per-expert dispatch lists (a token appears at most once per expert).
    idx_e, w_e = [], []
    for e in range(N_EXPERTS):
        a = np.nonzero(i1 == e)[0]
        b = np.nonzero(i2 == e)[0]
        idx = np.concatenate([a, b])
        w = np.concatenate([s1[a], s2[b]]).astype(np.float32)
        idx_e.append(idx)
        w_e.append(w)

    x2_bf = x2.astype(BF16)
    in_maps = []
    overflow = []  # (expert, token_ids, weights) beyond capacity
    for e in range(N_EXPERTS):
        idx = idx_e[e]
        if len(idx) > CAPACITY:
            overflow.append((e, idx[CAPACITY:], w_e[e][CAPACITY:]))
            idx = idx[:CAPACITY]
            idx_e[e] = idx
            w_e[e] = w_e[e][:CAPACITY]
        in_maps.append(_stage_expert(x2_bf, idx, W1[e], b1[e], W2[e], b2[e]))

    nc = _get_nc()
    res = None
    for attempt in range(3):  # transient NRT device errors: retry
        try:
            res = run_bass_kernel_spmd(
                nc, in_maps, core_ids=list(range(N_EXPERTS)), trace=TRACE
            )
            break
        except Exception:
            if attempt == 2:
                raise
            import time
            time.sleep(2.0)
    LAST_EXEC_NS = res.exec_time_ns
    if res.instructions_and_trace is not None:
        LAST_TRACE_PATH = res.instructions_and_trace[1]

    out = np.zeros((T, D), dtype=np.float32)
    for e in range(N_EXPERTS):
        idx = idx_e[e]
        if len(idx) == 0:
            continue
        ye = res.results[e]["yt"][:, :len(idx)].T.astype(np.float32)  # [n_e, D]
        out[idx] += w_e[e][:, None] * ye
    for e, idx, w in overflow:
        ye = _host_ffn_f64(x2[idx], W1[e], b1[e], W2[e], b2[e])
        out[idx] += (w[:, None] * ye).astype(np.float32)

    return out.reshape(B, S, D)
